# revision 15
# baseline (speedup 1.0000x reference)
"""HGCN decoder on 8 trn2 NeuronCores.

Strategy: nodes are sorted by in-degree, grouped into 128-node tiles, and the
tiles are dealt round-robin across the 8 cores (graph-parallel by destination
node).  Per layer each core publishes its tangent-space messages, AllGathers
the full [32768, 64] table to DRAM, then aggregates messages with `dma_gather`
(padded per-tile CSR) followed by a strided reduce on the vector engine.

This version software-pipelines all node-wise math INTO the gather phase
(tile groups of 4: gather -> reduce -> relu -> matmul -> scale overlap the
remaining gathers' DMA time), and collapses the hyperbolic scalar chains via
the identity artanh(min(tanh(min(y,15)), 1-eps)) == min(y, artanh(1-eps)),
so each layer's per-node math is just norms, min, reciprocal and multiplies
(no tanh/ln tables except one final tanh for the readout).  The layer
pipeline never materializes the on-manifold point: with u the tangent input,
r = relu-part and M = r @ W^T, the published tangent message is simply
min(nm * min(nu, A)/nu, A)/nm * M  (A = artanh(1-4e-3)).

All graph preprocessing (permutation, padded neighbor tables, weight folding
of edge/node masks, input transpose, first-layer input norms) happens
host-side in numpy; the device only sees dense tables.
"""

import numpy as np

N = 32768
E = 1015808
D = 64
C = 8          # cores
NL = N // C    # 4096 nodes per core
P = 128        # partitions / tile
T = NL // P    # 32 tiles per core
G = 4          # tile group for the pipeline / batched scalar chains
MAXN = 1.0 - 4e-3   # PROJ_EPS boundary for c=1
ART_MAXN = 3.1063030478757595   # artanh(1 - 4e-3)
MAX_TANH = 15.0
SQ_BIAS = 1e-12     # norm = sqrt(n2 + SQ_BIAS), replaces max(norm, EPS)


def _build_tables(rows, cols, edge_mask, node_mask):
    """Permute nodes by degree, deal tiles round-robin to cores, and build the
    per-core padded gather tables (int16 indices wrapped the way
    InstDMAGatherAnt wants them) plus matching weight tables."""
    deg = np.bincount(rows, minlength=N)
    order = np.argsort(-deg, kind="stable")
    # global tile j -> core j%C, slot j//C ; permuted position of its p-th node
    perm = np.empty(N, dtype=np.int64)
    j = np.arange(N) // P                     # global tile of sorted rank r
    c = j % C
    # slot relabel: degree-sorted slot s goes to program slot (s+1)%T so the
    # gather phase starts AND ends on a small-K tile (fast ramp, short tail)
    t = (j // C + 1) % T
    p = np.arange(N) % P
    perm[c * NL + t * P + p] = order          # perm[g] = original node id
    pos = np.empty(N, dtype=np.int64)
    pos[perm] = np.arange(N)

    # gather-table row id for permuted position g=(c,t,p):
    #   AllGather concatenates per-core [P, T*D] blocks, so
    #   row_id = c*NL + p*T + t
    gg = np.arange(N)
    gc, gr = gg // NL, gg % NL
    gt, gp_ = gr // P, gr % P
    rowid = gc * NL + gp_ * T + gt            # [g] -> table row
    dstpos = pos[rows]
    eorder = np.argsort(dstpos, kind="stable")
    src_sorted = rowid[pos[cols[eorder]]]     # gather table rows, 0..N-1
    w_sorted = edge_mask[eorder, 0].astype(np.float64)
    cnts = np.bincount(dstpos, minlength=N)
    offs = np.zeros(N + 1, dtype=np.int64)
    np.cumsum(cnts, out=offs[1:])

    # per-slot K: max count over the 8 cores' tiles in that slot
    cnts_g = cnts.reshape(C, T, P)
    Ks = np.maximum(cnts_g.max(axis=(0, 2)), 1).astype(np.int64)   # [T]

    IDXC = int(8 * Ks.sum())
    WTC = int(Ks.sum())
    idx_dev = np.zeros((C, P, IDXC), np.int16)
    wt_dev = np.zeros((C, P, WTC), np.float32)
    nm = node_mask[:, 0].astype(np.float64)
    ioff = woff = 0
    ar = None
    for t in range(T):
        K = int(Ks[t])
        if ar is None or ar.shape[1] != K:
            ar = np.arange(K)[None, :]
        for cc in range(C):
            base = cc * NL + t * P
            cn = cnts[base:base + P]
            take = offs[base:base + P][:, None] + ar          # [P, K]
            valid = ar < cn[:, None]
            take_c = np.minimum(take, E - 1)
            nb = np.where(valid, src_sorted[take_c], 0)
            wl = np.where(valid, w_sorted[take_c], 0.0)
            wl = wl * nm[perm[base:base + P]][:, None]
            il = nb.T.reshape(-1)                             # i = g*128+p
            ch = il.reshape(8 * K, 16).T                      # [16, 8K]
            idx_dev[cc, :, ioff:ioff + 8 * K] = np.tile(ch, (8, 1)).astype(np.int16)
            wt_dev[cc, :, woff:woff + K] = wl.astype(np.float32)
        ioff += 8 * K
        woff += K
    # pad counts per (core, slot, partition) for the pad-subtract path
    pc_dev = np.zeros((C, 1, T * P), np.float32)
    for t in range(T):
        K = int(Ks[t])
        for cc in range(C):
            base = cc * NL + t * P
            pc_dev[cc, 0, t * P:(t + 1) * P] = K - cnts[base:base + P]
    allones = bool(np.all(edge_mask == 1.0) and np.all(node_mask == 1.0))
    return perm, Ks, idx_dev, wt_dev, IDXC, WTC, pc_dev, allones


def _build_program(Ks, IDXC, WTC, use_wt=True, sim=False):
    import concourse.bacc as bacc
    import concourse.bass as bass
    import concourse.mybir as mybir
    import concourse.tile as tile
    from concourse import library_config
    from concourse.masks import make_identity

    f32 = mybir.dt.float32
    i16 = mybir.dt.int16
    AF = mybir.ActivationFunctionType
    OP = mybir.AluOpType
    X = mybir.AxisListType.X
    A = ART_MAXN
    A2 = ART_MAXN * ART_MAXN
    EPS2 = 1e-24

    nc = bacc.Bacc("TRN2", target_bir_lowering=False, debug=False,
                   num_devices=1 if sim else C)

    ht_in = nc.dram_tensor("ht_in", [D, NL], f32, kind="ExternalInput")
    t1h_in = nc.dram_tensor("t1h_in", [P, T], f32, kind="ExternalInput")
    idx_in = nc.dram_tensor("idx_in", [P, IDXC], i16, kind="ExternalInput")
    wt_in = nc.dram_tensor("wt_in", [P, WTC], f32, kind="ExternalInput")
    w0t_in = nc.dram_tensor("w0t_in", [D, D], f32, kind="ExternalInput")
    w1t_in = nc.dram_tensor("w1t_in", [D, D], f32, kind="ExternalInput")
    wot_in = nc.dram_tensor("wot_in", [D, 16], f32, kind="ExternalInput")
    pc_in = nc.dram_tensor("pc_in", [1, T * P], f32, kind="ExternalInput")
    out_dram = nc.dram_tensor("out", [P, T * 16], f32, kind="ExternalOutput")
    xt_shard = nc.dram_tensor("xt_shard", [P, T * D], f32)
    xt_table = nc.dram_tensor("xt_table", [N, D], f32, addr_space="Shared")
    groups = [list(range(C))]

    ioffs = np.zeros(T, dtype=np.int64)
    woffs = np.zeros(T, dtype=np.int64)
    np.cumsum(8 * Ks[:-1], out=ioffs[1:])
    np.cumsum(Ks[:-1], out=woffs[1:])

    with tile.TileContext(nc) as tc:
        nc.gpsimd.load_library(library_config.mlp)
        import contextlib
        ctx = contextlib.ExitStack()
        with ctx:
            const = ctx.enter_context(tc.tile_pool(name="const", bufs=1))
            sqp = ctx.enter_context(tc.tile_pool(name="sq", bufs=2))
            gp = ctx.enter_context(tc.tile_pool(name="gp", bufs=4))
            aggp = ctx.enter_context(tc.tile_pool(name="agg", bufs=3))
            rtp = ctx.enter_context(tc.tile_pool(name="rtp", bufs=4))
            scp = ctx.enter_context(tc.tile_pool(name="scp", bufs=2))
            psmv = ctx.enter_context(tc.tile_pool(name="psmv", bufs=4, space="PSUM"))
            psT = ctx.enter_context(tc.tile_pool(name="psT", bufs=3, space="PSUM"))
            psc = ctx.enter_context(tc.tile_pool(name="psc", bufs=1, space="PSUM"))

            ident = const.tile([P, P], f32)
            make_identity(nc, ident[:])
            # load order: head needs ht+w0t+t1h first; idx only at first gather
            ht_sb = const.tile([D, NL], f32)
            nc.sync.dma_start(out=ht_sb[:], in_=ht_in[:])
            w0t_sb = const.tile([D, D], f32)
            nc.sync.dma_start(out=w0t_sb[:], in_=w0t_in[:])
            t1h_sb = const.tile([P, T], f32)
            nc.sync.dma_start(out=t1h_sb[:], in_=t1h_in[:])
            idx_sb = const.tile([P, IDXC], i16)
            nc.sync.dma_start(out=idx_sb[:], in_=idx_in[:])
            w1t_sb = const.tile([D, D], f32)
            nc.sync.dma_start(out=w1t_sb[:], in_=w1t_in[:])
            wot_sb = const.tile([D, 16], f32)
            nc.sync.dma_start(out=wot_sb[:], in_=wot_in[:])
            pc_sb = const.tile([1, T * P], f32)
            nc.sync.dma_start(out=pc_sb[:], in_=pc_in[:])
            if use_wt:
                wt_sb = const.tile([P, WTC], f32)
                nc.sync.dma_start(out=wt_sb[:], in_=wt_in[:])

            sqb = const.tile([P, 1], f32)            # bias inside sqrt
            nc.gpsimd.memset(sqb[:], SQ_BIAS)
            msg_sb = const.tile([P, T * D], f32)     # published tangent msgs
            out_sb = const.tile([P, T * 16], f32)
            # per-node scalar accumulators / chain temps, one column per tile
            na2 = const.tile([P, T], f32)
            nr2 = const.tile([P, T], f32)
            nm2 = const.tile([P, T], f32)
            nA = const.tile([P, T], f32)
            nR = const.tile([P, T], f32)
            nM = const.tile([P, T], f32)
            rX = const.tile([P, T], f32)
            s2 = const.tile([P, T], f32)
            nu = const.tile([P, T], f32)
            tmp = const.tile([P, T], f32)
            t1 = const.tile([P, T], f32)
            yv = const.tile([P, T], f32)
            sc = const.tile([P, T], f32)
            cth = const.tile([P, T], f32)            # min(nu,15) for final tanh
            rRs = const.tile([P, T], f32)            # 1/nr for final scale

            def ts(t, w=D):
                return slice(t * w, (t + 1) * w)

            def gs(g0, w=D):
                return slice(g0 * w, (g0 + G) * w)

            def bc(ap_pt, w=D):
                # [P, G] column slice broadcast to [P, G, w] via stride-0 dim
                return bass.AP(ap_pt.tensor, ap_pt.offset,
                               list(ap_pt.ap) + [[0, w]])

            def r3(ap_2d, w=D):
                return ap_2d.rearrange("p (t d) -> p t d", d=w)

            def publish_slice(g0):
                # sim: the AllGather stand-in is written slice-wise, fully
                # overlapped with the gather phase; real: stage the shard
                # slice for the collective
                if sim:
                    nc.sync.dma_start(
                        out=xt_table[0:NL, :].rearrange(
                            "(p t) d -> p t d", p=P)[:, g0:g0 + G, :],
                        in_=r3(msg_sb[:, gs(g0)]))
                else:
                    nc.sync.dma_start(out=xt_shard[:, gs(g0)],
                                      in_=msg_sb[:, gs(g0)])

            def publish_table():
                tc.strict_bb_all_engine_barrier()
                if not sim:
                    nc.gpsimd.collective_compute(
                        "AllGather", mybir.AluOpType.bypass,
                        replica_groups=groups,
                        ins=[xt_shard[:, :]], outs=[xt_table[:, :]])
                tc.strict_bb_all_engine_barrier()

            # ---- head: msg0 = min(nm*t1h, A)/nm * (h @ W0^T) ----------------
            for g0 in range(0, T, G):
                sl = slice(g0, g0 + G)
                mvg = psmv.tile([P, G * D], f32, tag="mv")
                for t in range(g0, g0 + G):
                    nc.tensor.matmul(out=mvg[:, ts(t - g0)],
                                     lhsT=ht_sb[:, t * P:(t + 1) * P],
                                     rhs=w0t_sb[:], start=True, stop=True)
                sqg = sqp.tile([P, G * D], f32, tag="sq")
                nc.scalar.activation(sqg[:], mvg[:], AF.Square)
                nc.vector.tensor_reduce(nm2[:, sl], r3(sqg[:]), axis=X,
                                        op=OP.add)
                nc.scalar.activation(nM[:, sl], nm2[:, sl], AF.Sqrt, bias=sqb[:])
                nc.vector.tensor_tensor(yv[:, sl], nM[:, sl], t1h_sb[:, sl],
                                        op=OP.mult)
                nc.vector.reciprocal(rX[:, sl], nM[:, sl])
                nc.vector.tensor_scalar_min(tmp[:, sl], yv[:, sl], A)
                nc.vector.tensor_tensor(sc[:, sl], tmp[:, sl], rX[:, sl],
                                        op=OP.mult)
                nc.vector.tensor_tensor(r3(msg_sb[:, gs(g0)]),
                                        r3(mvg[:]),
                                        bc(sc[:, sl]), op=OP.mult)
                publish_slice(g0)
            publish_table()

            # ---- two gather phases: layer-1 messages, then readout ----------
            for phase in range(2):
                produce = phase == 0
                w_rhs = w1t_sb if produce else wot_sb
                if not use_wt:
                    row0_sb = scp.tile([1, D], f32, tag="row0")
                    nc.sync.dma_start(out=row0_sb[:], in_=xt_table[0:1, :])
                for g0 in range(0, T, G):
                    sl = slice(g0, g0 + G)
                    agrp = aggp.tile([P, G * D], f32, tag="agg")
                    for t in range(g0, g0 + G):
                        K = int(Ks[t])
                        gbuf = gp.tile([P, K * D], f32, tag="G")
                        g3 = gbuf[:].rearrange("p (k d) -> p k d", d=D)
                        nc.gpsimd.dma_gather(
                            g3, xt_table[:, :],
                            idx_sb[:, int(ioffs[t]):int(ioffs[t]) + 8 * K],
                            num_idxs=P * K, num_idxs_reg=P * K, elem_size=D,
                            single_packet=False)
                        if use_wt:
                            wt_ap = wt_sb[:, int(woffs[t]):int(woffs[t]) + K]
                            nc.vector.tensor_tensor(g3, g3, bc(wt_ap),
                                                    op=OP.mult)
                        nc.vector.tensor_reduce(
                            agrp[:, ts(t - g0)],
                            gbuf[:].rearrange("p (k d) -> p d k", d=D),
                            axis=X, op=OP.add)
                        if not use_wt:
                            corr = psc.tile([P, D], f32, tag="corr")
                            nc.tensor.matmul(
                                out=corr[:], lhsT=pc_sb[0:1, t * P:(t + 1) * P],
                                rhs=row0_sb[0:1, :], start=True, stop=True)
                            nc.vector.tensor_tensor(agrp[:, ts(t - g0)],
                                                    agrp[:, ts(t - g0)],
                                                    corr[:], op=OP.subtract)
                    # group-wide: na2, relu (in place), nr2
                    sqg = sqp.tile([P, G * D], f32, tag="sq")
                    nc.scalar.activation(sqg[:], agrp[:], AF.Square)
                    nc.vector.tensor_reduce(na2[:, sl], r3(sqg[:]), axis=X,
                                            op=OP.add)
                    nc.scalar.activation(agrp[:], agrp[:], AF.Relu)
                    sqg2 = sqp.tile([P, G * D], f32, tag="sq")
                    nc.scalar.activation(sqg2[:], agrp[:], AF.Square)
                    nc.vector.tensor_reduce(nr2[:, sl], r3(sqg2[:]), axis=X,
                                            op=OP.add)
                    if produce:
                        mvg = psmv.tile([P, G * D], f32, tag="mv")
                        w_mv = D
                    else:
                        mvg = psmv.tile([P, G * 16], f32, tag="mv")
                        w_mv = 16
                        # full readout chain in-phase: gam applied to r itself
                        # (out = (gam*r) @ Wo^T), so nothing remains at the end
                        nc.vector.tensor_scalar_add(nA[:, sl], na2[:, sl], EPS2)
                        nc.vector.reciprocal(rX[:, sl], nA[:, sl])
                        nc.vector.tensor_scalar_min(tmp[:, sl], nA[:, sl], A2)
                        nc.vector.tensor_tensor(s2[:, sl], tmp[:, sl],
                                                rX[:, sl], op=OP.mult)
                        nc.vector.tensor_tensor(nu[:, sl], s2[:, sl],
                                                nr2[:, sl], op=OP.mult)
                        nc.vector.tensor_scalar_min(cth[:, sl], nu[:, sl],
                                                    MAX_TANH * MAX_TANH)
                        nc.vector.tensor_scalar_add(nR[:, sl], nr2[:, sl], EPS2)
                        nc.vector.reciprocal(rRs[:, sl], nR[:, sl])
                        nc.scalar.activation(nu[:, sl], cth[:, sl], AF.Sqrt)
                        nc.scalar.activation(nR[:, sl], rRs[:, sl], AF.Sqrt)
                        # tanh(x) ~ x*(945+105x^2+x^4)/(945+420x^2+15x^4),
                        # x = min(nu,15), x^2 = cth; error <5e-4 before the
                        # MAXN clip
                        nc.vector.tensor_scalar_add(t1[:, sl], cth[:, sl], 105.0)
                        nc.vector.tensor_tensor(t1[:, sl], t1[:, sl],
                                                cth[:, sl], op=OP.mult)
                        nc.vector.tensor_scalar_add(t1[:, sl], t1[:, sl], 945.0)
                        nc.vector.tensor_scalar(yv[:, sl], cth[:, sl], 15.0,
                                                420.0, op0=OP.mult, op1=OP.add)
                        nc.vector.tensor_tensor(yv[:, sl], yv[:, sl],
                                                cth[:, sl], op=OP.mult)
                        nc.vector.tensor_scalar_add(yv[:, sl], yv[:, sl], 945.0)
                        nc.vector.reciprocal(yv[:, sl], yv[:, sl])
                        nc.vector.tensor_tensor(t1[:, sl], t1[:, sl],
                                                yv[:, sl], op=OP.mult)
                        nc.vector.tensor_tensor(t1[:, sl], t1[:, sl],
                                                nu[:, sl], op=OP.mult)
                        nc.vector.tensor_scalar_min(t1[:, sl], t1[:, sl], MAXN)
                        nc.vector.tensor_tensor(sc[:, sl], t1[:, sl],
                                                nR[:, sl], op=OP.mult)
                        nc.vector.tensor_tensor(r3(agrp[:]), r3(agrp[:]),
                                                bc(sc[:, sl]), op=OP.mult)
                    for t in range(g0, g0 + G):
                        rT_ps = psT.tile([D, P], f32, tag="rT")
                        nc.tensor.transpose(out=rT_ps[:], in_=agrp[:, ts(t - g0)],
                                            identity=ident[:])
                        rT = rtp.tile([D, P], f32, tag="rTs")
                        nc.scalar.copy(rT[:], rT_ps[:])
                        nc.tensor.matmul(out=mvg[:, ts(t - g0, w_mv)], lhsT=rT[:],
                                         rhs=w_rhs[:], start=True, stop=True)
                    if produce:
                        sqg3 = sqp.tile([P, G * D], f32, tag="sq")
                        nc.scalar.activation(sqg3[:], mvg[:], AF.Square)
                        nc.vector.tensor_reduce(nm2[:, sl], r3(sqg3[:]), axis=X,
                                                op=OP.add)
                        # squared-norm chain: sc^2 = min(y2,A2)/nm2 with
                        # y2 = nm2*min(s2^2*nr2, A2)/nr2, s2^2 = min(na2,A2)/na2
                        nc.vector.tensor_scalar_add(nA[:, sl], na2[:, sl], EPS2)
                        nc.vector.reciprocal(rX[:, sl], nA[:, sl])
                        nc.vector.tensor_scalar_min(tmp[:, sl], nA[:, sl], A2)
                        nc.vector.tensor_tensor(s2[:, sl], tmp[:, sl],
                                                rX[:, sl], op=OP.mult)
                        nc.vector.tensor_tensor(nu[:, sl], s2[:, sl],
                                                nr2[:, sl], op=OP.mult)
                        nc.vector.tensor_scalar_add(nR[:, sl], nr2[:, sl], EPS2)
                        nc.vector.reciprocal(rX[:, sl], nR[:, sl])
                        nc.vector.tensor_scalar_min(tmp[:, sl], nu[:, sl], A2)
                        nc.vector.tensor_tensor(t1[:, sl], tmp[:, sl],
                                                rX[:, sl], op=OP.mult)
                        nc.vector.tensor_tensor(yv[:, sl], nm2[:, sl],
                                                t1[:, sl], op=OP.mult)
                        nc.vector.tensor_scalar_add(nM[:, sl], nm2[:, sl], EPS2)
                        nc.vector.reciprocal(rX[:, sl], nM[:, sl])
                        nc.vector.tensor_scalar_min(tmp[:, sl], yv[:, sl], A2)
                        nc.vector.tensor_tensor(tmp[:, sl], tmp[:, sl],
                                                rX[:, sl], op=OP.mult)
                        nc.scalar.activation(sc[:, sl], tmp[:, sl], AF.Sqrt,
                                             bias=sqb[:])
                        nc.vector.tensor_tensor(r3(msg_sb[:, gs(g0)]),
                                                r3(mvg[:]),
                                                bc(sc[:, sl]), op=OP.mult)
                        publish_slice(g0)
                    else:
                        nc.vector.tensor_copy(out_sb[:, gs(g0, 16)],
                                              mvg[:])
                        nc.sync.dma_start(out=out_dram[:, gs(g0, 16)],
                                          in_=out_sb[:, gs(g0, 16)])
                if produce:
                    publish_table()

    nc.compile()
    return nc


def kernel(h, distances, rows, cols, node_mask, edge_mask,
           W0, b0, W1, b1, W_out, b_out, _trace=False):
    from concourse.bass_utils import run_bass_kernel_spmd

    h = np.asarray(h, dtype=np.float32)
    rows = np.asarray(rows).astype(np.int64)
    cols = np.asarray(cols).astype(np.int64)
    node_mask = np.asarray(node_mask, dtype=np.float32)
    edge_mask = np.asarray(edge_mask, dtype=np.float32)
    assert not np.any(np.asarray(b0)) and not np.any(np.asarray(b1)) and \
        not np.any(np.asarray(b_out)), "nonzero biases unsupported"

    perm, Ks, idx_dev, wt_dev, IDXC, WTC, pc_dev, allones = _build_tables(
        rows, cols, edge_mask, node_mask)

    hp = h[perm].reshape(C, T, P, D)
    ht = np.ascontiguousarray(hp.transpose(0, 3, 1, 2).reshape(C, D, NL))
    nh = np.sqrt((hp.astype(np.float64) ** 2).sum(-1))          # [C, T, P]
    t1h = (np.minimum(nh, ART_MAXN) / np.maximum(nh, 1e-15))
    t1h = np.ascontiguousarray(t1h.transpose(0, 2, 1)).astype(np.float32)
    w0t = np.ascontiguousarray(np.asarray(W0, np.float32).T)
    w1t = np.ascontiguousarray(np.asarray(W1, np.float32).T)
    wot = np.ascontiguousarray(np.asarray(W_out, np.float32).T)

    nc = _build_program(Ks, IDXC, WTC, use_wt=not allones)
    in_maps = [{
        "ht_in": ht[c],
        "t1h_in": t1h[c],
        "idx_in": idx_dev[c],
        "wt_in": wt_dev[c],
        "w0t_in": w0t, "w1t_in": w1t, "wot_in": wot,
        "pc_in": pc_dev[c],
    } for c in range(C)]
    res = run_bass_kernel_spmd(nc, in_maps, list(range(C)), trace=_trace)
    od = np.stack([res.results[c]["out"] for c in range(C)])
    od = od.reshape(C, P, T, 16).transpose(0, 2, 1, 3).reshape(N, 16)
    out = np.empty((N, 16), np.float32)
    out[perm] = od
    if _trace:
        return out, res
    return out
